# revision 1
# baseline (speedup 1.0000x reference)
"""Trainium2 Bass kernel for the caption-generation module (2-layer GRU
encoder-decoder + vocab projection + log_softmax).

Strategy: data-parallel over batch across 8 NeuronCores (B=128 -> 16 rows
per core, weights replicated).  Per core, everything runs in a transposed
layout (feature dim on SBUF partitions, (time*batch) on the free dim):

  E1:  gi1[t] = x_t @ w_ih1.T for all 40 encoder steps  (one batched matmul)
  C1:  h1 chain, 67 sequential steps, only h1 @ w_hh1.T inside the loop
       (decoder rnn1 input is zero so its gi is just the bias)
  E3:  gi2[t] = [h1_t; w_t] @ w_ih2.T for all 67 steps  (batched matmul)
  C2:  h2 chain, 67 sequential steps
  P :  logits = h2_dec @ out_w.T + out_b, then streamed log_softmax,
       DMA straight to the output

Matmul inputs are cast to bf16 (fp32 accumulate in PSUM); gate math and
softmax run in fp32.
"""

import sys
import types

sys.path.insert(0, "/opt/trn_rl_repo")

import numpy as np
import ml_dtypes

import concourse.bass as bass
import concourse.mybir as mybir
import concourse.tile as tile
from concourse.alu_op_type import AluOpType
from concourse.vector_clock import ScopedClock

BF16 = mybir.dt.bfloat16
F32 = mybir.dt.float32
F8 = mybir.dt.float8e3
WSCALE = 128.0  # fp8 chain-weight pre-scale (host multiplies, gates divide)
AF = mybir.ActivationFunctionType


# ---------------------------------------------------------------------------
# Workaround: this container's walrus rejects CTRL instructions carrying more
# than one sync-wait command.  Split the TileContext tail drain's wait list
# across a chain of drains, one wait each.
# ---------------------------------------------------------------------------
def _patched_drain_and_barrier(self, tick_clock, wait_clock):
    import bass_rust

    drain_inst = self.nc.sync.drain()
    wait_clock.add_sem_waits(
        drain_inst.ins, ScopedClock({None: tick_clock.global_clock})
    )
    waits = list(drain_inst.ins.sync_info.on_wait)
    if len(waits) > 1:
        si = drain_inst.ins.sync_info
        si.on_wait = waits[:1]
        drain_inst.ins.sync_info = si
        for i in range(1, len(waits)):
            extra = self.nc.sync.drain()
            extra.ins.sync_info = bass_rust.SyncInfo(
                on_wait=waits[i : i + 1], on_update=[]
            )
    self.nc.all_engine_barrier()
    assert self.sems is not None
    popped = self.nc._tile_sem_poison_stack.pop()
    assert popped is self._sem_poison
    self.nc.clear_and_free_semaphores(list(self.sems.allocated().values()))
    self.nc.all_engine_barrier()


tile.TileContext._drain_and_barrier = _patched_drain_and_barrier

# Same walrus limitation for regular engine instructions: at most one
# sync-wait per instruction.  Split extra waits onto preceding NoOps on the
# same engine (engine stalls there instead — identical semantics).
_orig_commit = tile.TileContext._commit_instruction


def _commit_split_waits(self, inst, lazy_reg_writes=True):
    si = getattr(inst, "sync_info", None)
    if (si is not None and si.on_wait and len(si.on_wait) > 1
            and inst.engine != mybir.EngineType.Unassigned):
        waits = list(si.on_wait)
        for w in waits[:-1]:
            nop = mybir.InstNoOp(
                name=self.nc.get_next_instruction_name(),
                sync_info=mybir.SyncInfo(on_wait=[w], on_update=[]),
                bass_nofuse=True,
                engine=inst.engine,
            )
            _orig_commit(self, nop, lazy_reg_writes=False)
        si.on_wait = waits[-1:]
        inst.sync_info = si
    return _orig_commit(self, inst, lazy_reg_writes)


tile.TileContext._commit_instruction = _commit_split_waits


# ---------------------------------------------------------------------------
# Config
# ---------------------------------------------------------------------------
def make_cfg(B=128, NF=40, TD=27, V=16000, DV=2048, DH=512, DW=512,
             n_cores=8, has_out_b=False, chain_mode="fp32"):
    cfg = dict(B=B, NF=NF, TD=TD, V=V, DV=DV, DH=DH, DW=DW,
               n_cores=n_cores, has_out_b=has_out_b, chain_mode=chain_mode)
    cfg["BS"] = B // n_cores          # batch rows per core
    cfg["KV"] = DV // 128             # x feature chunks
    cfg["KH"] = DH // 128             # h feature chunks
    cfg["KW"] = DW // 128             # word feature chunks
    cfg["MC"] = 3 * DH // 128         # gate chunks
    cfg["NSTEP"] = NF + TD            # total chain steps
    cfg["ROWS_E"] = NF * cfg["BS"]    # encoder (t,b) columns
    cfg["ROWS_A"] = cfg["NSTEP"] * cfg["BS"]
    cfg["ROWS_D"] = TD * cfg["BS"]    # decode (t,b) columns
    # vocab tiling for the projection (psum free dim <= 512 fp32)
    for pn in (512, 500, 400, 320, 256):
        if V % pn == 0:
            cfg["PN"] = pn
            break
    else:
        raise ValueError(f"V={V} has no tile size")
    cfg["VCH"] = V // 4               # log_softmax streaming chunk
    return cfg


def _ntiles(total, maxn):
    """Split `total` into tiles of at most maxn (last ragged)."""
    out = []
    n0 = 0
    while n0 < total:
        nn = min(maxn, total - n0)
        out.append((n0, nn))
        n0 += nn
    return out


# ---------------------------------------------------------------------------
# Kernel builder
# ---------------------------------------------------------------------------
def build_nc(cfg):
    BS, KV, KH, KW, MC = cfg["BS"], cfg["KV"], cfg["KH"], cfg["KW"], cfg["MC"]
    NF, TD, V, DH = cfg["NF"], cfg["TD"], cfg["V"], cfg["DH"]
    NSTEP, ROWS_E, ROWS_D = cfg["NSTEP"], cfg["ROWS_E"], cfg["ROWS_D"]
    PN, VCH = cfg["PN"], cfg["VCH"]
    G3 = 3 * DH
    LAG = 12  # h2 chain trails h1 by this many steps (> E3 block size)

    nc = bass.Bass()

    # ---- DRAM parameters (per-core views; host prepares these) ----
    xT = nc.dram_tensor("xT", [cfg["DV"], ROWS_E], BF16, kind="ExternalInput")
    wordsT = nc.dram_tensor("wordsT", [cfg["DW"], ROWS_D], BF16, kind="ExternalInput")
    w1T = nc.dram_tensor("w1T", [cfg["DV"], G3], BF16, kind="ExternalInput")
    chain_wdt = {"fp8": F8, "bf16": BF16, "fp32": F32}[cfg["chain_mode"]]
    chain_hdt = F32 if cfg["chain_mode"] == "fp32" else BF16
    wh1T = nc.dram_tensor("wh1T", [DH, G3], chain_wdt, kind="ExternalInput")
    w2T = nc.dram_tensor("w2T", [DH + cfg["DW"], G3], BF16, kind="ExternalInput")
    wh2T = nc.dram_tensor("wh2T", [DH, G3], chain_wdt, kind="ExternalInput")
    owT = nc.dram_tensor("owT", [DH, V], BF16, kind="ExternalInput")
    bi1c = nc.dram_tensor("bi1c", [128, MC], F32, kind="ExternalInput")
    bi2c = nc.dram_tensor("bi2c", [128, MC], F32, kind="ExternalInput")
    gidec = nc.dram_tensor("gidec", [128, MC, BS], F32, kind="ExternalInput")
    bhn1 = nc.dram_tensor("bhn1", [128, KH, BS], F32, kind="ExternalInput")
    bhn2 = nc.dram_tensor("bhn2", [128, KH, BS], F32, kind="ExternalInput")
    if cfg["has_out_b"]:
        outb = nc.dram_tensor("outb", [1, V], BF16, kind="ExternalInput")
        ones = nc.dram_tensor("ones", [1, 128], BF16, kind="ExternalInput")
    out = nc.dram_tensor("out", [BS, TD, V], F32, kind="ExternalOutput")
    # view [t, b, v] of out[b, t, v] (strides V, TD*V, 1); row r = t*BS + b
    _o = out[:]
    out_tbv = bass.AP(tensor=_o.tensor, offset=_o.offset,
                      ap=[[V, TD], [TD * V, BS], [1, V]])

    def out_slice(r0, mrows, c0, cw):
        assert r0 % BS == 0 and mrows % BS == 0
        return out_tbv[r0 // BS:(r0 + mrows) // BS, :, c0:c0 + cw]

    from contextlib import ExitStack

    with tile.TileContext(nc) as tc:
        with (
            tc.tile_pool(name="pconst", bufs=1) as pconst,
            tc.tile_pool(name="pchain", bufs=6) as pchain,
            tc.tile_pool(name="psum_mm", bufs=3, space="PSUM") as psum_mm,
            tc.tile_pool(name="psum_gh", bufs=5, space="PSUM") as psum_gh,
            tc.tile_pool(name="ph2", bufs=1) as ph2,
        ):
            # ---- constants ----
            bi1c_sb = pconst.tile([128, MC], F32, tag="bi1c")
            nc.sync.dma_start(out=bi1c_sb[:], in_=bi1c[:])
            bi2c_sb = pconst.tile([128, MC], F32, tag="bi2c")
            nc.sync.dma_start(out=bi2c_sb[:], in_=bi2c[:])
            gidec_sb = pconst.tile([128, MC, BS], F32, tag="gidec")
            nc.sync.dma_start(out=gidec_sb[:], in_=gidec[:])
            bhn1_sb = pconst.tile([128, KH, BS], F32, tag="bhn1")
            nc.sync.dma_start(out=bhn1_sb[:], in_=bhn1[:])
            bhn2_sb = pconst.tile([128, KH, BS], F32, tag="bhn2")
            nc.sync.dma_start(out=bhn2_sb[:], in_=bhn2[:])
            if cfg["has_out_b"]:
                outb_sb = pconst.tile([1, V], BF16, tag="outb")
                nc.sync.dma_start(out=outb_sb[:], in_=outb[:])
                ones_sb = pconst.tile([1, 128], BF16, tag="ones")
                nc.sync.dma_start(out=ones_sb[:], in_=ones[:])

            h2_sb = ph2.tile([128, KH, (NSTEP + 1) * BS], chain_hdt, tag="h2")
            nc.vector.memset(h2_sb[:, :, 0:BS], 0.0)
            if cfg["chain_mode"] == "fp32":
                h2b_sb = ph2.tile([128, KH, (NSTEP + 1) * BS], BF16, tag="h2b")
                nc.vector.memset(h2b_sb[:, :, 0:BS], 0.0)
            else:
                h2b_sb = h2_sb

            # ---------------- gate math shared by both chains -------------
            def gru_step(t, save_sb, gh, gi, bhh, shadow_sb=None):
                """gh: [128, MC, BS] psum (already = W_hh @ h).
                gi: [128, MC, BS] (includes b_ih, and b_hh for r/z chunks).
                save_sb holds hT; block t is h_{t-1}, writes block t+1."""
                prev = save_sb[:, :, t * BS:(t + 1) * BS]
                ws = (1.0 / WSCALE) if cfg["chain_mode"] == "fp8" else 1.0
                tmpn = pchain.tile([128, KH, BS], F32, tag="tmpn")
                nc.vector.scalar_tensor_tensor(
                    out=tmpn[:], in0=gh[:, 2 * KH:, :], scalar=ws, in1=bhh[:],
                    op0=AluOpType.mult, op1=AluOpType.add)
                rz = pchain.tile([128, 2 * KH, BS], F32, tag="rz")
                nc.vector.scalar_tensor_tensor(
                    out=rz[:], in0=gh[:, 0:2 * KH, :], scalar=ws,
                    in1=gi[:, 0:2 * KH, :],
                    op0=AluOpType.mult, op1=AluOpType.add)
                rzs = pchain.tile([128, 2 * KH, BS], F32, tag="rzs")
                nc.scalar.activation(out=rzs[:], in_=rz[:], func=AF.Sigmoid)
                np0 = pchain.tile([128, KH, BS], F32, tag="np0")
                nc.vector.tensor_tensor(
                    out=np0[:], in0=rzs[:, 0:KH, :], in1=tmpn[:], op=AluOpType.mult)
                np1 = pchain.tile([128, KH, BS], F32, tag="np1")
                nc.vector.tensor_tensor(
                    out=np1[:], in0=gi[:, 2 * KH:, :], in1=np0[:], op=AluOpType.add)
                nt = pchain.tile([128, KH, BS], F32, tag="nt")
                nc.scalar.activation(out=nt[:], in_=np1[:], func=AF.Tanh)
                hm0 = pchain.tile([128, KH, BS], F32, tag="hm0")
                nc.vector.tensor_tensor(
                    out=hm0[:], in0=prev[:], in1=nt[:], op=AluOpType.subtract)
                hm1 = pchain.tile([128, KH, BS], F32, tag="hm1")
                nc.vector.tensor_tensor(
                    out=hm1[:], in0=rzs[:, KH:, :], in1=hm0[:], op=AluOpType.mult)
                nc.vector.tensor_tensor(
                    out=save_sb[:, :, (t + 1) * BS:(t + 2) * BS],
                    in0=nt[:], in1=hm1[:], op=AluOpType.add)
                if shadow_sb is not None:
                    nc.vector.tensor_copy(
                        out=shadow_sb[:, :, (t + 1) * BS:(t + 2) * BS],
                        in_=save_sb[:, :, (t + 1) * BS:(t + 2) * BS])

            def recur_matmul(whh_sb, save_sb, t):
                gh = psum_gh.tile([128, MC, BS], F32, tag="gh")
                prev = save_sb[:, :, t * BS:(t + 1) * BS]
                for m in range(MC):
                    for k in range(KH):
                        nc.tensor.matmul(
                            gh[:, m, :],
                            lhsT=whh_sb[:, k, m * 128:(m + 1) * 128],
                            rhs=prev[:, k, :],
                            start=(k == 0), stop=(k == KH - 1))
                return gh

            # ====== E1, interleaved h1/E3/h2 chains, projection ======
            with ExitStack() as chain_es:
                pmidA = chain_es.enter_context(tc.tile_pool(name="pmidA", bufs=1))
                h1_sb = pmidA.tile([128, KH, (NSTEP + 1) * BS], chain_hdt, tag="h1")
                nc.vector.memset(h1_sb[:, :, 0:BS], 0.0)
                if cfg["chain_mode"] == "fp32":
                    h1b_sb = pmidA.tile([128, KH, (NSTEP + 1) * BS], BF16,
                                        tag="h1b")
                    nc.vector.memset(h1b_sb[:, :, 0:BS], 0.0)
                else:
                    h1b_sb = h1_sb
                gi1_sb = pmidA.tile([128, MC, ROWS_E], BF16, tag="gi1")
                wh1_sb = pmidA.tile([128, KH, G3], chain_wdt, tag="wh1")

                with tc.tile_pool(name="pw1", bufs=1) as pw1:
                    x_sb = pw1.tile([128, KV, ROWS_E], BF16, tag="x")
                    xT_r = xT[:].rearrange("(k p) n -> p k n", p=128)
                    for k in range(KV):
                        nc.sync.dma_start(out=x_sb[:, k, :], in_=xT_r[:, k, :])
                    w1_sb = pw1.tile([128, KV, G3], BF16, tag="w1")
                    w1T_r = w1T[:].rearrange("(k p) n -> p k n", p=128)
                    for k in range(KV):
                        nc.sync.dma_start(out=w1_sb[:, k, :], in_=w1T_r[:, k, :])
                    wh1T_r = wh1T[:].rearrange("(k p) n -> p k n", p=128)
                    for k in range(KH):
                        nc.sync.dma_start(out=wh1_sb[:, k, :], in_=wh1T_r[:, k, :])

                    # E1: gi1 = w1T.T @ x  (+ bias via ACT copy)
                    for (n0, nn) in _ntiles(ROWS_E, 320):
                        for m in range(MC):
                            ps = psum_mm.tile([128, 512], F32, tag="mm")
                            for k in range(KV):
                                nc.tensor.matmul(
                                    ps[:, :nn],
                                    lhsT=w1_sb[:, k, m * 128:(m + 1) * 128],
                                    rhs=x_sb[:, k, n0:n0 + nn],
                                    start=(k == 0), stop=(k == KV - 1))
                            nc.scalar.activation(
                                out=gi1_sb[:, m, n0:n0 + nn], in_=ps[:, :nn],
                                func=AF.Identity, bias=bi1c_sb[:, m:m + 1],
                                scale=1.0)

                # layer-2 weights / words / gi2 (loaded while chains run)
                pmidB = chain_es.enter_context(tc.tile_pool(name="pmidB", bufs=1))
                w2_sb = pmidB.tile([128, KH + KW, G3], BF16, tag="w2")
                w2T_r = w2T[:].rearrange("(k p) n -> p k n", p=128)
                for k in range(KH + KW):
                    nc.sync.dma_start(out=w2_sb[:, k, :], in_=w2T_r[:, k, :])
                words_sb = pmidB.tile([128, KW, ROWS_D], BF16, tag="words")
                wordsT_r = wordsT[:].rearrange("(k p) n -> p k n", p=128)
                for k in range(KW):
                    nc.sync.dma_start(out=words_sb[:, k, :], in_=wordsT_r[:, k, :])
                wh2_sb = pmidB.tile([128, KH, G3], chain_wdt, tag="wh2")
                wh2T_r = wh2T[:].rearrange("(k p) n -> p k n", p=128)
                for k in range(KH):
                    nc.sync.dma_start(out=wh2_sb[:, k, :], in_=wh2T_r[:, k, :])
                gi2_sb = pmidB.tile([128, MC, NSTEP * BS], BF16, tag="gi2")

                def h1_step(t):
                    gh = recur_matmul(wh1_sb, h1_sb, t)
                    gi = (gi1_sb[:, :, t * BS:(t + 1) * BS] if t < NF
                          else gidec_sb[:])
                    gru_step(t, h1_sb, gh, gi, bhn1_sb,
                             h1b_sb if h1b_sb is not h1_sb else None)

                def h2_step(t):
                    gh = recur_matmul(wh2_sb, h2_sb, t)
                    gru_step(t, h2_sb, gh,
                             gi2_sb[:, :, t * BS:(t + 1) * BS], bhn2_sb,
                             h2b_sb if h2b_sb is not h2_sb else None)

                def e3_block(t0, nsteps):
                    """gi2 for chain steps [t0, t0+nsteps)."""
                    n0 = t0 * BS
                    nn = nsteps * BS
                    enc = t0 < NF  # blocks never straddle NF
                    for m in range(MC):
                        ps = psum_mm.tile([128, 512], F32, tag="mm")
                        for k in range(KH):
                            nc.tensor.matmul(
                                ps[:, :nn],
                                lhsT=w2_sb[:, k, m * 128:(m + 1) * 128],
                                rhs=h1b_sb[:, k, BS + n0:BS + n0 + nn],
                                start=(k == 0),
                                stop=(enc and k == KH - 1))
                        if not enc:
                            w0 = n0 - ROWS_E
                            for k in range(KW):
                                nc.tensor.matmul(
                                    ps[:, :nn],
                                    lhsT=w2_sb[:, KH + k, m * 128:(m + 1) * 128],
                                    rhs=words_sb[:, k, w0:w0 + nn],
                                    start=False, stop=(k == KW - 1))
                        nc.scalar.activation(
                            out=gi2_sb[:, m, n0:n0 + nn], in_=ps[:, :nn],
                            func=AF.Identity, bias=bi2c_sb[:, m:m + 1],
                            scale=1.0)

                # step-granularity interleave: each engine's in-order stream
                # alternates h1[t] / h2[t-LAG] so one chain's stalls are
                # filled by the other's ready work.
                blocks = ([(t0, nn) for (t0, nn) in _ntiles(NF, 8)] +
                          [(NF + t0, nn) for (t0, nn) in _ntiles(TD, 9)])
                block_end = {t0 + nn: (t0, nn) for (t0, nn) in blocks}
                for tt in range(NSTEP + LAG):
                    if tt < NSTEP:
                        h1_step(tt)
                        if tt + 1 in block_end:
                            e3_block(*block_end[tt + 1])
                    s = tt - LAG
                    if 0 <= s < NSTEP:
                        h2_step(s)
            # ---- projection + log_softmax over decode steps ----
            # |logits| is bounded well below fp32 exp overflow here, so
            # log_softmax runs without the max shift: lp = x - ln(sum(e^x)).
            # Vocab-tile outer loop: each out_w slice is DMA'd exactly once;
            # all four 128-row logit tiles stay live (bf16).
            with (
                tc.tile_pool(name="pp", bufs=1) as pp,
                tc.tile_pool(name="pwst", bufs=3) as pwst,
                tc.tile_pool(name="pstage", bufs=2) as pstage,
                tc.tile_pool(name="psmall", bufs=2) as psmall,
            ):
                owT_r = owT[:].rearrange("(k p) n -> p k n", p=128)
                nvt = V // PN
                dcol0 = (NF + 1) * BS  # first decode h2 col
                mtiles = _ntiles(ROWS_D, 128)
                logits_t = [pp.tile([128, V], BF16, tag=f"logits{i}",
                                    name=f"logits{i}")
                            for i in range(len(mtiles))]
                sums_t = [psmall.tile([128, nvt], F32, tag=f"sums{i}",
                                      name=f"sums{i}")
                          for i in range(len(mtiles))]
                for nt_i in range(nvt):
                    n0 = nt_i * PN
                    wst = pwst.tile([128, KH, PN], BF16, tag="wst")
                    nc.sync.dma_start(out=wst[:], in_=owT_r[:, :, n0:n0 + PN])
                    for mt, (r0, mrows) in enumerate(mtiles):
                        ps = psum_mm.tile([128, 512], F32, tag="mm")
                        last = KH - 1 if not cfg["has_out_b"] else None
                        for k in range(KH):
                            nc.tensor.matmul(
                                ps[:mrows, :PN],
                                lhsT=h2b_sb[:, k, dcol0 + r0:dcol0 + r0 + mrows],
                                rhs=wst[:, k, :],
                                start=(k == 0), stop=(k == last))
                        if cfg["has_out_b"]:
                            nc.tensor.matmul(
                                ps[:mrows, :PN],
                                lhsT=ones_sb[:, :mrows],
                                rhs=outb_sb[:, n0:n0 + PN],
                                start=False, stop=True)
                        edump = pstage.tile([128, PN], BF16, tag="edump")
                        nc.scalar.activation(
                            out=edump[:mrows, :], in_=ps[:mrows, :PN],
                            func=AF.Exp,
                            accum_out=sums_t[mt][:mrows, nt_i:nt_i + 1])
                        nc.vector.tensor_copy(
                            out=logits_t[mt][:mrows, n0:n0 + PN],
                            in_=ps[:mrows, :PN])
                # tail: lse per row tile, then logp = logits - lse,
                # alternating ACT / DVE per chunk, output via SWDGE
                NCH = 8
                CW = V // NCH
                for mt, (r0, mrows) in enumerate(mtiles):
                    s1 = psmall.tile([128, 1], F32, tag="s1")
                    nc.vector.tensor_reduce(
                        out=s1[:mrows], in_=sums_t[mt][:mrows, :],
                        axis=mybir.AxisListType.X, op=AluOpType.add)
                    nshift = psmall.tile([128, 1], F32, tag="nshift")
                    nc.scalar.activation(
                        out=nshift[:mrows], in_=s1[:mrows], func=AF.Ln)
                    nc.vector.tensor_scalar_mul(
                        nshift[:mrows], nshift[:mrows], -1.0)
                    for c in range(NCH):
                        stage = pstage.tile([128, CW], F32, tag="stage")
                        src = logits_t[mt][:mrows, c * CW:(c + 1) * CW]
                        if c % 2 == 0:
                            nc.scalar.activation(
                                out=stage[:mrows, :], in_=src,
                                func=AF.Identity, bias=nshift[:mrows])
                        else:
                            nc.vector.tensor_scalar_add(
                                stage[:mrows, :], src, nshift[:mrows])
                        nc.gpsimd.dma_start(
                            out=out_slice(r0, mrows, c * CW, CW),
                            in_=stage[:mrows, :])
    return nc



# ---------------------------------------------------------------------------
# Host side
# ---------------------------------------------------------------------------
def _bf16(a):
    return np.ascontiguousarray(a, dtype=np.float32).astype(ml_dtypes.bfloat16)


def _f32(a):
    return np.ascontiguousarray(a, dtype=np.float32)


def prep_inputs(cfg, vid_feats, target_variable, emb, w_ih1, w_hh1, b_ih1,
                b_hh1, w_ih2, w_hh2, b_ih2, b_hh2, out_w, out_b):
    """Build per-core input maps."""
    BS, MC, KH, DH = cfg["BS"], cfg["MC"], cfg["KH"], cfg["DH"]
    TD, NC = cfg["TD"], cfg["n_cores"]

    vid_feats = np.asarray(vid_feats, dtype=np.float32)
    target_variable = np.asarray(target_variable)
    emb = np.asarray(emb, dtype=np.float32)

    # replicated tensors
    if cfg["chain_mode"] == "fp8":
        def _chain_w(a):
            f8max = float(ml_dtypes.finfo(ml_dtypes.float8_e3m4).max)
            scaled = np.clip(np.asarray(a, dtype=np.float32) * WSCALE,
                             -f8max, f8max)
            return np.ascontiguousarray(scaled).astype(ml_dtypes.float8_e3m4)
    elif cfg["chain_mode"] == "fp32":
        _chain_w = _f32
    else:
        _chain_w = _bf16
    shared = {
        "w1T": _bf16(np.asarray(w_ih1).T),
        "wh1T": _chain_w(np.asarray(w_hh1).T),
        "w2T": _bf16(np.asarray(w_ih2).T),
        "wh2T": _chain_w(np.asarray(w_hh2).T),
        "owT": _bf16(np.asarray(out_w).T),
    }
    # combined biases: b_ih (+ b_hh for the r,z chunks; the n chunk of b_hh
    # is applied inside the gate, before the r multiply)
    def comb(bi, bh):
        c = np.asarray(bi, dtype=np.float32).copy()
        c[: 2 * DH] += np.asarray(bh, dtype=np.float32)[: 2 * DH]
        return c

    c1 = comb(b_ih1, b_hh1)
    c2 = comb(b_ih2, b_hh2)
    shared["bi1c"] = _f32(c1.reshape(MC, 128).T)
    shared["bi2c"] = _f32(c2.reshape(MC, 128).T)
    shared["gidec"] = _f32(
        np.broadcast_to(c1.reshape(MC, 128).T[:, :, None], (128, MC, BS)))
    shared["bhn1"] = _f32(np.broadcast_to(
        np.asarray(b_hh1, np.float32)[2 * DH:].reshape(KH, 128).T[:, :, None],
        (128, KH, BS)))
    shared["bhn2"] = _f32(np.broadcast_to(
        np.asarray(b_hh2, np.float32)[2 * DH:].reshape(KH, 128).T[:, :, None],
        (128, KH, BS)))
    if cfg["has_out_b"]:
        shared["outb"] = _bf16(np.asarray(out_b).reshape(1, -1))
        shared["ones"] = _bf16(np.ones((1, 128)))

    words = emb[np.asarray(target_variable[:, :TD], dtype=np.int64)]  # [B,TD,DW]

    in_maps = []
    for c in range(NC):
        sl = slice(c * BS, (c + 1) * BS)
        vs = vid_feats[sl]                      # [BS, NF, DV]
        ws = words[sl]                          # [BS, TD, DW]
        m = dict(shared)
        m["xT"] = _bf16(vs.transpose(2, 1, 0).reshape(cfg["DV"], -1))
        m["wordsT"] = _bf16(ws.transpose(2, 1, 0).reshape(cfg["DW"], -1))
        in_maps.append(m)
    return in_maps


_CACHE = {}
LAST_RESULT = None


def kernel(**inputs):
    global LAST_RESULT
    from concourse.bass_utils import run_bass_kernel_spmd

    out_b = np.asarray(inputs["out_b"])
    has_out_b = bool(np.any(out_b))
    key = ("full", has_out_b)
    if key not in _CACHE:
        cfg = make_cfg(has_out_b=has_out_b)
        _CACHE[key] = (cfg, build_nc(cfg))
    cfg, nc = _CACHE[key]

    in_maps = prep_inputs(cfg, **inputs)
    res = run_bass_kernel_spmd(nc, in_maps, list(range(cfg["n_cores"])))
    LAST_RESULT = res
    outs = [res.results[c]["out"] for c in range(cfg["n_cores"])]
    return np.concatenate(outs, axis=0)  # [B, TD, V]



# revision 2
# speedup vs baseline: 3.7139x; 3.7139x over previous
"""Trainium2 Bass kernel for the caption-generation module (2-layer GRU
encoder-decoder + vocab projection + log_softmax).

Strategy: data-parallel over batch across 8 NeuronCores (B=128 -> 16 rows
per core, weights replicated).  Per core, everything runs in a transposed
layout (feature dim on SBUF partitions, (time*batch) on the free dim):

  E1:  gi1[t] = x_t @ w_ih1.T for all 40 encoder steps  (one batched matmul)
  C1:  h1 chain, 67 sequential steps, only h1 @ w_hh1.T inside the loop
       (decoder rnn1 input is zero so its gi is just the bias)
  E3:  gi2[t] = [h1_t; w_t] @ w_ih2.T for all 67 steps  (batched matmul)
  C2:  h2 chain, 67 sequential steps
  P :  logits = h2_dec @ out_w.T + out_b, then streamed log_softmax,
       DMA straight to the output

Matmul inputs are cast to bf16 (fp32 accumulate in PSUM); gate math and
softmax run in fp32.
"""

import sys
import types

sys.path.insert(0, "/opt/trn_rl_repo")

import numpy as np
import ml_dtypes

import concourse.bass as bass
import concourse.mybir as mybir
import concourse.tile as tile
from concourse.alu_op_type import AluOpType
from concourse.vector_clock import ScopedClock

BF16 = mybir.dt.bfloat16
F32 = mybir.dt.float32
F8 = mybir.dt.float8e3
WSCALE = 128.0  # fp8 chain-weight pre-scale (host multiplies, gates divide)
AF = mybir.ActivationFunctionType


# ---------------------------------------------------------------------------
# Workaround: this container's walrus rejects CTRL instructions carrying more
# than one sync-wait command.  Split the TileContext tail drain's wait list
# across a chain of drains, one wait each.
# ---------------------------------------------------------------------------
def _patched_drain_and_barrier(self, tick_clock, wait_clock):
    import bass_rust

    drain_inst = self.nc.sync.drain()
    wait_clock.add_sem_waits(
        drain_inst.ins, ScopedClock({None: tick_clock.global_clock})
    )
    waits = list(drain_inst.ins.sync_info.on_wait)
    if len(waits) > 1:
        si = drain_inst.ins.sync_info
        si.on_wait = waits[:1]
        drain_inst.ins.sync_info = si
        for i in range(1, len(waits)):
            extra = self.nc.sync.drain()
            extra.ins.sync_info = bass_rust.SyncInfo(
                on_wait=waits[i : i + 1], on_update=[]
            )
    self.nc.all_engine_barrier()
    assert self.sems is not None
    popped = self.nc._tile_sem_poison_stack.pop()
    assert popped is self._sem_poison
    self.nc.clear_and_free_semaphores(list(self.sems.allocated().values()))
    self.nc.all_engine_barrier()


tile.TileContext._drain_and_barrier = _patched_drain_and_barrier

# Same walrus limitation for regular engine instructions: at most one
# sync-wait per instruction.  Split extra waits onto preceding NoOps on the
# same engine (engine stalls there instead — identical semantics).
_orig_commit = tile.TileContext._commit_instruction


def _commit_split_waits(self, inst, lazy_reg_writes=True):
    si = getattr(inst, "sync_info", None)
    if (si is not None and si.on_wait and len(si.on_wait) > 1
            and inst.engine != mybir.EngineType.Unassigned):
        waits = list(si.on_wait)
        for w in waits[:-1]:
            nop = mybir.InstNoOp(
                name=self.nc.get_next_instruction_name(),
                sync_info=mybir.SyncInfo(on_wait=[w], on_update=[]),
                bass_nofuse=True,
                engine=inst.engine,
            )
            _orig_commit(self, nop, lazy_reg_writes=False)
        si.on_wait = waits[-1:]
        inst.sync_info = si
    return _orig_commit(self, inst, lazy_reg_writes)


tile.TileContext._commit_instruction = _commit_split_waits


# ---------------------------------------------------------------------------
# Config
# ---------------------------------------------------------------------------
def make_cfg(B=128, NF=40, TD=27, V=16000, DV=2048, DH=512, DW=512,
             n_cores=8, has_out_b=False, chain_mode="fp32"):
    cfg = dict(B=B, NF=NF, TD=TD, V=V, DV=DV, DH=DH, DW=DW,
               n_cores=n_cores, has_out_b=has_out_b, chain_mode=chain_mode)
    cfg["BS"] = B // n_cores          # batch rows per core
    cfg["KV"] = DV // 128             # x feature chunks
    cfg["KH"] = DH // 128             # h feature chunks
    cfg["KW"] = DW // 128             # word feature chunks
    cfg["MC"] = 3 * DH // 128         # gate chunks
    cfg["NSTEP"] = NF + TD            # total chain steps
    cfg["ROWS_E"] = NF * cfg["BS"]    # encoder (t,b) columns
    cfg["ROWS_A"] = cfg["NSTEP"] * cfg["BS"]
    cfg["ROWS_D"] = TD * cfg["BS"]    # decode (t,b) columns
    # vocab tiling for the projection (psum free dim <= 512 fp32)
    for pn in (512, 500, 400, 320, 256):
        if V % pn == 0:
            cfg["PN"] = pn
            break
    else:
        raise ValueError(f"V={V} has no tile size")
    cfg["VCH"] = V // 4               # log_softmax streaming chunk
    return cfg


def _ntiles(total, maxn):
    """Split `total` into tiles of at most maxn (last ragged)."""
    out = []
    n0 = 0
    while n0 < total:
        nn = min(maxn, total - n0)
        out.append((n0, nn))
        n0 += nn
    return out


# ---------------------------------------------------------------------------
# Kernel builder
# ---------------------------------------------------------------------------
def build_nc(cfg):
    BS, KV, KH, KW, MC = cfg["BS"], cfg["KV"], cfg["KH"], cfg["KW"], cfg["MC"]
    NF, TD, V, DH = cfg["NF"], cfg["TD"], cfg["V"], cfg["DH"]
    NSTEP, ROWS_E, ROWS_D = cfg["NSTEP"], cfg["ROWS_E"], cfg["ROWS_D"]
    PN, VCH = cfg["PN"], cfg["VCH"]
    G3 = 3 * DH
    LAG = 12  # h2 chain trails h1 by this many steps (> E3 block size)

    nc = bass.Bass()

    # ---- DRAM parameters (per-core views; host prepares these) ----
    xT = nc.dram_tensor("xT", [cfg["DV"], ROWS_E], BF16, kind="ExternalInput")
    wordsT = nc.dram_tensor("wordsT", [cfg["DW"], ROWS_D], BF16, kind="ExternalInput")
    w1T = nc.dram_tensor("w1T", [cfg["DV"], G3], BF16, kind="ExternalInput")
    chain_wdt = {"fp8": F8, "bf16": BF16, "fp32": F32}[cfg["chain_mode"]]
    chain_hdt = F32 if cfg["chain_mode"] == "fp32" else BF16
    wh1T = nc.dram_tensor("wh1T", [DH, G3], chain_wdt, kind="ExternalInput")
    w2T = nc.dram_tensor("w2T", [DH + cfg["DW"], G3], BF16, kind="ExternalInput")
    wh2T = nc.dram_tensor("wh2T", [DH, G3], chain_wdt, kind="ExternalInput")
    owT = nc.dram_tensor("owT", [DH, V], BF16, kind="ExternalInput")
    bi1c = nc.dram_tensor("bi1c", [128, MC], F32, kind="ExternalInput")
    bi2c = nc.dram_tensor("bi2c", [128, MC], F32, kind="ExternalInput")
    gidec = nc.dram_tensor("gidec", [128, MC, BS], F32, kind="ExternalInput")
    bhn1 = nc.dram_tensor("bhn1", [128, KH, BS], F32, kind="ExternalInput")
    bhn2 = nc.dram_tensor("bhn2", [128, KH, BS], F32, kind="ExternalInput")
    if cfg["has_out_b"]:
        outb = nc.dram_tensor("outb", [1, V], BF16, kind="ExternalInput")
        ones = nc.dram_tensor("ones", [1, 128], BF16, kind="ExternalInput")
    out = nc.dram_tensor("out", [BS, TD, V], F32, kind="ExternalOutput")
    # view [t, b, v] of out[b, t, v] (strides V, TD*V, 1); row r = t*BS + b
    _o = out[:]
    out_tbv = bass.AP(tensor=_o.tensor, offset=_o.offset,
                      ap=[[V, TD], [TD * V, BS], [1, V]])

    def out_slice(r0, mrows, c0, cw):
        assert r0 % BS == 0 and mrows % BS == 0
        return out_tbv[r0 // BS:(r0 + mrows) // BS, :, c0:c0 + cw]

    from contextlib import ExitStack

    with tile.TileContext(nc) as tc:
        with (
            tc.tile_pool(name="pconst", bufs=1) as pconst,
            tc.tile_pool(name="pchain", bufs=6) as pchain,
            tc.tile_pool(name="psum_mm", bufs=3, space="PSUM") as psum_mm,
            tc.tile_pool(name="psum_gh", bufs=5, space="PSUM") as psum_gh,
            tc.tile_pool(name="ph2", bufs=1) as ph2,
        ):
            # ---- constants ----
            bi1c_sb = pconst.tile([128, MC], F32, tag="bi1c")
            nc.sync.dma_start(out=bi1c_sb[:], in_=bi1c[:])
            bi2c_sb = pconst.tile([128, MC], F32, tag="bi2c")
            nc.sync.dma_start(out=bi2c_sb[:], in_=bi2c[:])
            gidec_sb = pconst.tile([128, MC, BS], F32, tag="gidec")
            nc.sync.dma_start(out=gidec_sb[:], in_=gidec[:])
            bhn1_sb = pconst.tile([128, KH, BS], F32, tag="bhn1")
            nc.sync.dma_start(out=bhn1_sb[:], in_=bhn1[:])
            bhn2_sb = pconst.tile([128, KH, BS], F32, tag="bhn2")
            nc.sync.dma_start(out=bhn2_sb[:], in_=bhn2[:])
            if cfg["has_out_b"]:
                outb_sb = pconst.tile([1, V], BF16, tag="outb")
                nc.sync.dma_start(out=outb_sb[:], in_=outb[:])
                ones_sb = pconst.tile([1, 128], BF16, tag="ones")
                nc.sync.dma_start(out=ones_sb[:], in_=ones[:])

            h2_sb = ph2.tile([128, KH, (NSTEP + 1) * BS], chain_hdt, tag="h2")
            nc.vector.memset(h2_sb[:, :, 0:BS], 0.0)
            if cfg["chain_mode"] == "fp32":
                h2b_sb = ph2.tile([128, KH, (NSTEP + 1) * BS], BF16, tag="h2b")
                nc.vector.memset(h2b_sb[:, :, 0:BS], 0.0)
            else:
                h2b_sb = h2_sb

            # ---------------- gate math shared by both chains -------------
            def gru_step(t, save_sb, gh, gi, bhh, shadow_sb=None):
                """gh: [128, MC, BS] psum (already = W_hh @ h).
                gi: [128, MC, BS] (includes b_ih, and b_hh for r/z chunks).
                save_sb holds hT; block t is h_{t-1}, writes block t+1."""
                prev = save_sb[:, :, t * BS:(t + 1) * BS]
                ws = (1.0 / WSCALE) if cfg["chain_mode"] == "fp8" else 1.0
                tmpn = pchain.tile([128, KH, BS], F32, tag="tmpn")
                nc.vector.scalar_tensor_tensor(
                    out=tmpn[:], in0=gh[:, 2 * KH:, :], scalar=ws, in1=bhh[:],
                    op0=AluOpType.mult, op1=AluOpType.add)
                rz = pchain.tile([128, 2 * KH, BS], F32, tag="rz")
                nc.vector.scalar_tensor_tensor(
                    out=rz[:], in0=gh[:, 0:2 * KH, :], scalar=ws,
                    in1=gi[:, 0:2 * KH, :],
                    op0=AluOpType.mult, op1=AluOpType.add)
                rzs = pchain.tile([128, 2 * KH, BS], F32, tag="rzs")
                nc.scalar.activation(out=rzs[:], in_=rz[:], func=AF.Sigmoid)
                np0 = pchain.tile([128, KH, BS], F32, tag="np0")
                nc.vector.tensor_tensor(
                    out=np0[:], in0=rzs[:, 0:KH, :], in1=tmpn[:], op=AluOpType.mult)
                np1 = pchain.tile([128, KH, BS], F32, tag="np1")
                nc.vector.tensor_tensor(
                    out=np1[:], in0=gi[:, 2 * KH:, :], in1=np0[:], op=AluOpType.add)
                nt = pchain.tile([128, KH, BS], F32, tag="nt")
                nc.scalar.activation(out=nt[:], in_=np1[:], func=AF.Tanh)
                hm0 = pchain.tile([128, KH, BS], F32, tag="hm0")
                nc.vector.tensor_tensor(
                    out=hm0[:], in0=prev[:], in1=nt[:], op=AluOpType.subtract)
                hm1 = pchain.tile([128, KH, BS], F32, tag="hm1")
                nc.vector.tensor_tensor(
                    out=hm1[:], in0=rzs[:, KH:, :], in1=hm0[:], op=AluOpType.mult)
                nc.vector.tensor_tensor(
                    out=save_sb[:, :, (t + 1) * BS:(t + 2) * BS],
                    in0=nt[:], in1=hm1[:], op=AluOpType.add)
                if shadow_sb is not None:
                    nc.vector.tensor_copy(
                        out=shadow_sb[:, :, (t + 1) * BS:(t + 2) * BS],
                        in_=save_sb[:, :, (t + 1) * BS:(t + 2) * BS])

            def recur_matmul(whh_sb, save_sb, t):
                gh = psum_gh.tile([128, MC, BS], F32, tag="gh")
                prev = save_sb[:, :, t * BS:(t + 1) * BS]
                for m in range(MC):
                    for k in range(KH):
                        nc.tensor.matmul(
                            gh[:, m, :],
                            lhsT=whh_sb[:, k, m * 128:(m + 1) * 128],
                            rhs=prev[:, k, :],
                            start=(k == 0), stop=(k == KH - 1))
                return gh

            # ====== E1, interleaved h1/E3/h2 chains, projection ======
            with ExitStack() as chain_es:
                pmidA = chain_es.enter_context(tc.tile_pool(name="pmidA", bufs=1))
                h1_sb = pmidA.tile([128, KH, (NSTEP + 1) * BS], chain_hdt, tag="h1")
                nc.vector.memset(h1_sb[:, :, 0:BS], 0.0)
                if cfg["chain_mode"] == "fp32":
                    h1b_sb = pmidA.tile([128, KH, (NSTEP + 1) * BS], BF16,
                                        tag="h1b")
                    nc.vector.memset(h1b_sb[:, :, 0:BS], 0.0)
                else:
                    h1b_sb = h1_sb
                gi1_sb = pmidA.tile([128, MC, ROWS_E], BF16, tag="gi1")
                wh1_sb = pmidA.tile([128, KH, G3], chain_wdt, tag="wh1")

                with tc.tile_pool(name="pw1", bufs=1) as pw1:
                    x_sb = pw1.tile([128, KV, ROWS_E], BF16, tag="x")
                    xT_r = xT[:].rearrange("(k p) n -> p k n", p=128)
                    for k in range(KV):
                        nc.sync.dma_start(out=x_sb[:, k, :], in_=xT_r[:, k, :])
                    w1_sb = pw1.tile([128, KV, G3], BF16, tag="w1")
                    w1T_r = w1T[:].rearrange("(k p) n -> p k n", p=128)
                    for k in range(KV):
                        nc.sync.dma_start(out=w1_sb[:, k, :], in_=w1T_r[:, k, :])
                    wh1T_r = wh1T[:].rearrange("(k p) n -> p k n", p=128)
                    for k in range(KH):
                        nc.sync.dma_start(out=wh1_sb[:, k, :], in_=wh1T_r[:, k, :])

                    # E1: gi1 = w1T.T @ x  (+ bias via ACT copy)
                    for (n0, nn) in _ntiles(ROWS_E, 320):
                        for m in range(MC):
                            ps = psum_mm.tile([128, 512], F32, tag="mm")
                            for k in range(KV):
                                nc.tensor.matmul(
                                    ps[:, :nn],
                                    lhsT=w1_sb[:, k, m * 128:(m + 1) * 128],
                                    rhs=x_sb[:, k, n0:n0 + nn],
                                    start=(k == 0), stop=(k == KV - 1))
                            nc.scalar.activation(
                                out=gi1_sb[:, m, n0:n0 + nn], in_=ps[:, :nn],
                                func=AF.Identity, bias=bi1c_sb[:, m:m + 1],
                                scale=1.0)

                # layer-2 weights / words / gi2 (loaded while chains run)
                pmidB = chain_es.enter_context(tc.tile_pool(name="pmidB", bufs=1))
                w2_sb = pmidB.tile([128, KH + KW, G3], BF16, tag="w2")
                w2T_r = w2T[:].rearrange("(k p) n -> p k n", p=128)
                for k in range(KH + KW):
                    nc.sync.dma_start(out=w2_sb[:, k, :], in_=w2T_r[:, k, :])
                words_sb = pmidB.tile([128, KW, ROWS_D], BF16, tag="words")
                wordsT_r = wordsT[:].rearrange("(k p) n -> p k n", p=128)
                for k in range(KW):
                    nc.sync.dma_start(out=words_sb[:, k, :], in_=wordsT_r[:, k, :])
                wh2_sb = pmidB.tile([128, KH, G3], chain_wdt, tag="wh2")
                wh2T_r = wh2T[:].rearrange("(k p) n -> p k n", p=128)
                for k in range(KH):
                    nc.sync.dma_start(out=wh2_sb[:, k, :], in_=wh2T_r[:, k, :])
                gi2_sb = pmidB.tile([128, MC, NSTEP * BS], BF16, tag="gi2")

                def h1_step(t):
                    gh = recur_matmul(wh1_sb, h1_sb, t)
                    gi = (gi1_sb[:, :, t * BS:(t + 1) * BS] if t < NF
                          else gidec_sb[:])
                    gru_step(t, h1_sb, gh, gi, bhn1_sb,
                             h1b_sb if h1b_sb is not h1_sb else None)

                def h2_step(t):
                    gh = recur_matmul(wh2_sb, h2_sb, t)
                    gru_step(t, h2_sb, gh,
                             gi2_sb[:, :, t * BS:(t + 1) * BS], bhn2_sb,
                             h2b_sb if h2b_sb is not h2_sb else None)

                def e3_block(t0, nsteps):
                    """gi2 for chain steps [t0, t0+nsteps)."""
                    n0 = t0 * BS
                    nn = nsteps * BS
                    enc = t0 < NF  # blocks never straddle NF
                    for m in range(MC):
                        ps = psum_mm.tile([128, 512], F32, tag="mm")
                        for k in range(KH):
                            nc.tensor.matmul(
                                ps[:, :nn],
                                lhsT=w2_sb[:, k, m * 128:(m + 1) * 128],
                                rhs=h1b_sb[:, k, BS + n0:BS + n0 + nn],
                                start=(k == 0),
                                stop=(enc and k == KH - 1))
                        if not enc:
                            w0 = n0 - ROWS_E
                            for k in range(KW):
                                nc.tensor.matmul(
                                    ps[:, :nn],
                                    lhsT=w2_sb[:, KH + k, m * 128:(m + 1) * 128],
                                    rhs=words_sb[:, k, w0:w0 + nn],
                                    start=False, stop=(k == KW - 1))
                        nc.scalar.activation(
                            out=gi2_sb[:, m, n0:n0 + nn], in_=ps[:, :nn],
                            func=AF.Identity, bias=bi2c_sb[:, m:m + 1],
                            scale=1.0)

                # step-granularity interleave: each engine's in-order stream
                # alternates h1[t] / h2[t-LAG] so one chain's stalls are
                # filled by the other's ready work.
                blocks = ([(t0, nn) for (t0, nn) in _ntiles(NF, 8)] +
                          [(NF + t0, nn) for (t0, nn) in _ntiles(TD, 9)])
                block_end = {t0 + nn: (t0, nn) for (t0, nn) in blocks}
                for tt in range(NSTEP + LAG):
                    if tt < NSTEP:
                        h1_step(tt)
                        if tt + 1 in block_end:
                            e3_block(*block_end[tt + 1])
                    s = tt - LAG
                    if 0 <= s < NSTEP:
                        h2_step(s)
            # ---- projection + log_softmax over decode steps ----
            # |logits| is bounded well below fp32 exp overflow here, so
            # log_softmax runs without the max shift: lp = x - ln(sum(e^x)).
            # Vocab-tile outer loop: each out_w slice is DMA'd exactly once;
            # all four 128-row logit tiles stay live (bf16).
            with (
                tc.tile_pool(name="pp", bufs=1) as pp,
                tc.tile_pool(name="pwst", bufs=3) as pwst,
                tc.tile_pool(name="pstage", bufs=2) as pstage,
                tc.tile_pool(name="psmall", bufs=2) as psmall,
            ):
                owT_r = owT[:].rearrange("(k p) n -> p k n", p=128)
                nvt = V // PN
                dcol0 = (NF + 1) * BS  # first decode h2 col
                mtiles = _ntiles(ROWS_D, 128)
                logits_t = [pp.tile([128, V], BF16, tag=f"logits{i}",
                                    name=f"logits{i}")
                            for i in range(len(mtiles))]
                sums_t = [psmall.tile([128, nvt], F32, tag=f"sums{i}",
                                      name=f"sums{i}")
                          for i in range(len(mtiles))]
                for nt_i in range(nvt):
                    n0 = nt_i * PN
                    wst = pwst.tile([128, KH, PN], BF16, tag="wst")
                    nc.sync.dma_start(out=wst[:], in_=owT_r[:, :, n0:n0 + PN])
                    for mt, (r0, mrows) in enumerate(mtiles):
                        ps = psum_mm.tile([128, 512], F32, tag="mm")
                        last = KH - 1 if not cfg["has_out_b"] else None
                        for k in range(KH):
                            nc.tensor.matmul(
                                ps[:mrows, :PN],
                                lhsT=h2b_sb[:, k, dcol0 + r0:dcol0 + r0 + mrows],
                                rhs=wst[:, k, :],
                                start=(k == 0), stop=(k == last))
                        if cfg["has_out_b"]:
                            nc.tensor.matmul(
                                ps[:mrows, :PN],
                                lhsT=ones_sb[:, :mrows],
                                rhs=outb_sb[:, n0:n0 + PN],
                                start=False, stop=True)
                        edump = pstage.tile([128, PN], BF16, tag="edump")
                        nc.scalar.activation(
                            out=edump[:mrows, :], in_=ps[:mrows, :PN],
                            func=AF.Exp,
                            accum_out=sums_t[mt][:mrows, nt_i:nt_i + 1])
                        nc.vector.tensor_copy(
                            out=logits_t[mt][:mrows, n0:n0 + PN],
                            in_=ps[:mrows, :PN])
                # tail: lse per row tile, then logp = logits - lse,
                # alternating ACT / DVE per chunk, output via SWDGE
                NCH = 8
                CW = V // NCH
                for mt, (r0, mrows) in enumerate(mtiles):
                    s1 = psmall.tile([128, 1], F32, tag="s1")
                    nc.vector.tensor_reduce(
                        out=s1[:mrows], in_=sums_t[mt][:mrows, :],
                        axis=mybir.AxisListType.X, op=AluOpType.add)
                    nshift = psmall.tile([128, 1], F32, tag="nshift")
                    nc.scalar.activation(
                        out=nshift[:mrows], in_=s1[:mrows], func=AF.Ln)
                    nc.vector.tensor_scalar_mul(
                        nshift[:mrows], nshift[:mrows], -1.0)
                    for c in range(NCH):
                        stage = pstage.tile([128, CW], F32, tag="stage")
                        src = logits_t[mt][:mrows, c * CW:(c + 1) * CW]
                        if c % 2 == 0:
                            nc.scalar.activation(
                                out=stage[:mrows, :], in_=src,
                                func=AF.Identity, bias=nshift[:mrows])
                        else:
                            nc.vector.tensor_scalar_add(
                                stage[:mrows, :], src, nshift[:mrows])
                        nc.gpsimd.dma_start(
                            out=out_slice(r0, mrows, c * CW, CW),
                            in_=stage[:mrows, :])
    return nc



# ---------------------------------------------------------------------------
# Host side
# ---------------------------------------------------------------------------
def _bf16(a):
    return np.ascontiguousarray(a, dtype=np.float32).astype(ml_dtypes.bfloat16)


def _f32(a):
    return np.ascontiguousarray(a, dtype=np.float32)


def prep_inputs(cfg, vid_feats, target_variable, emb, w_ih1, w_hh1, b_ih1,
                b_hh1, w_ih2, w_hh2, b_ih2, b_hh2, out_w, out_b):
    """Build per-core input maps."""
    BS, MC, KH, DH = cfg["BS"], cfg["MC"], cfg["KH"], cfg["DH"]
    TD, NC = cfg["TD"], cfg["n_cores"]

    vid_feats = np.asarray(vid_feats, dtype=np.float32)
    target_variable = np.asarray(target_variable)
    emb = np.asarray(emb, dtype=np.float32)

    # replicated tensors
    if cfg["chain_mode"] == "fp8":
        def _chain_w(a):
            f8max = float(ml_dtypes.finfo(ml_dtypes.float8_e3m4).max)
            scaled = np.clip(np.asarray(a, dtype=np.float32) * WSCALE,
                             -f8max, f8max)
            return np.ascontiguousarray(scaled).astype(ml_dtypes.float8_e3m4)
    elif cfg["chain_mode"] == "fp32":
        _chain_w = _f32
    else:
        _chain_w = _bf16
    shared = {
        "w1T": _bf16(np.asarray(w_ih1).T),
        "wh1T": _chain_w(np.asarray(w_hh1).T),
        "w2T": _bf16(np.asarray(w_ih2).T),
        "wh2T": _chain_w(np.asarray(w_hh2).T),
        "owT": _bf16(np.asarray(out_w).T),
    }
    # combined biases: b_ih (+ b_hh for the r,z chunks; the n chunk of b_hh
    # is applied inside the gate, before the r multiply)
    def comb(bi, bh):
        c = np.asarray(bi, dtype=np.float32).copy()
        c[: 2 * DH] += np.asarray(bh, dtype=np.float32)[: 2 * DH]
        return c

    c1 = comb(b_ih1, b_hh1)
    c2 = comb(b_ih2, b_hh2)
    shared["bi1c"] = _f32(c1.reshape(MC, 128).T)
    shared["bi2c"] = _f32(c2.reshape(MC, 128).T)
    shared["gidec"] = _f32(
        np.broadcast_to(c1.reshape(MC, 128).T[:, :, None], (128, MC, BS)))
    shared["bhn1"] = _f32(np.broadcast_to(
        np.asarray(b_hh1, np.float32)[2 * DH:].reshape(KH, 128).T[:, :, None],
        (128, KH, BS)))
    shared["bhn2"] = _f32(np.broadcast_to(
        np.asarray(b_hh2, np.float32)[2 * DH:].reshape(KH, 128).T[:, :, None],
        (128, KH, BS)))
    if cfg["has_out_b"]:
        shared["outb"] = _bf16(np.asarray(out_b).reshape(1, -1))
        shared["ones"] = _bf16(np.ones((1, 128)))

    words = emb[np.asarray(target_variable[:, :TD], dtype=np.int64)]  # [B,TD,DW]

    in_maps = []
    for c in range(NC):
        sl = slice(c * BS, (c + 1) * BS)
        vs = vid_feats[sl]                      # [BS, NF, DV]
        ws = words[sl]                          # [BS, TD, DW]
        m = dict(shared)
        m["xT"] = _bf16(vs.transpose(2, 1, 0).reshape(cfg["DV"], -1))
        m["wordsT"] = _bf16(ws.transpose(2, 1, 0).reshape(cfg["DW"], -1))
        in_maps.append(m)
    return in_maps


_CACHE = {}
LAST_RESULT = None


CHAIN_MODE = "bf16"


def kernel(**inputs):
    global LAST_RESULT
    from concourse.bass_utils import run_bass_kernel_spmd

    out_b = np.asarray(inputs["out_b"])
    has_out_b = bool(np.any(out_b))
    key = ("full", has_out_b, CHAIN_MODE)
    if key not in _CACHE:
        cfg = make_cfg(has_out_b=has_out_b, chain_mode=CHAIN_MODE)
        _CACHE[key] = (cfg, build_nc(cfg))
    cfg, nc = _CACHE[key]

    in_maps = prep_inputs(cfg, **inputs)
    res = run_bass_kernel_spmd(nc, in_maps, list(range(cfg["n_cores"])))
    LAST_RESULT = res
    outs = [res.results[c]["out"] for c in range(cfg["n_cores"])]
    return np.concatenate(outs, axis=0)  # [B, TD, V]

